# revision 1
# baseline (speedup 1.0000x reference)
"""Multi-head attention (B=4, S=2048, D=1024, H=16) on 8 Trainium2 NeuronCores.

Sharding: core c handles batch c//2 and head-group c%2 (8 heads = 512 dims of
the per-head concat). Each core computes its q/k/v projections (tensor
parallel over heads), attention for its 8 heads, and a partial output
projection over its 512 concat dims; the host sums the two partials per batch.

Device dataflow (per core, all matmuls fp32r = full-rate PE):
  - qT/kT [d, s] layouts from projection (contraction over embedding on
    partitions), v in [s, d] layout with a fused ones-column per head.
  - scores computed transposed S^T[k, q] so the softmax mask/bias is a
    per-partition ACT bias and exp(scale*s + bias) is a single ACT op
    (no max subtraction needed: |scores| <= ~4 by construction).
  - ctx^T = [V | 1]^T @ P^T accumulated over k-chunks; row 64 of the psum is
    the softmax denominator (flash-style deferred normalization).
  - normalization: exact reciprocal of the denominator row, partition-broadcast
    via a DRAM bounce, one DVE multiply per head into ctx^T.
  - out^T partial = ctx_cat^T chunks @ Wo^T slices, streamed to DRAM.

Host epilogue: out[b] = partial[2b] + partial[2b+1] + (Wo @ bv + bo); the
value bias commutes with softmax (rows sum to 1) so it is exact. The key bias
is softmax-invariant (constant per query) and is still applied on-device for
exactness; so is the query bias.
"""

import sys

sys.path.insert(0, "/opt/trn_rl_repo")

import numpy as np

import concourse.bacc as bacc
import concourse.mybir as mybir
import concourse.tile as tile
from concourse.bass_utils import run_bass_kernel_spmd
import concourse.bass_utils as _bu

# Re-enable walrus LDWEIGHTS dedupe: fp32r matmuls are self-loading (no FWL,
# no background-buffer preload), so a repeated stationary operand otherwise
# pays a full serial weight load per matmul.
if False:
    _orig_run_command = _bu.run_command

    def _run_command_ldwopt(cmd, **kw):
        cmd = [
            "--enable-ldw-opt=true" if c == "--enable-ldw-opt=false" else c
            for c in cmd
        ]
        return _orig_run_command(cmd, **kw)

    _bu.run_command = _run_command_ldwopt
    _bu._ldwopt_patched = True

f32 = mybir.dt.float32
f32r = mybir.dt.float32r
bf16 = mybir.dt.bfloat16
AF = mybir.ActivationFunctionType

B, S, E, H = 4, 2048, 1024, 16
DH = E // H  # 64
G = E // 2  # 512 dims per core (8 heads)
HL = H // 2  # heads per core
EC = E // 128  # 8 e-chunks (projection contraction)
DC = G // 128  # 4 head-pairs per core
QT = S // 512  # 4 q-tiles
KC = S // 128  # 16 k-chunks
SC = S // 128  # 16 s-chunks (output rows)
GC = G // 128  # 4 chunks of the local concat dim (out-proj contraction)
SCALE = 1.0 / np.sqrt(np.float64(E))
MASK_NEG = -88.0  # exp(-88 + |s|max) == 0 in fp32 for masked keys

_NC = None


def _build_program():
    nc = bacc.Bacc("TRN2", target_bir_lowering=False, debug=False, num_devices=8)

    xqT = nc.dram_tensor("xqT", [E, S], bf16, kind="ExternalInput").ap()
    xkT = nc.dram_tensor("xkT", [E, S], bf16, kind="ExternalInput").ap()
    xvT = nc.dram_tensor("xvT", [E, S], f32, kind="ExternalInput").ap()
    wqT = nc.dram_tensor("wqT", [E, G], bf16, kind="ExternalInput").ap()
    wkT = nc.dram_tensor("wkT", [E, G], bf16, kind="ExternalInput").ap()
    wvT = nc.dram_tensor("wvT", [E, G], f32, kind="ExternalInput").ap()
    woT = nc.dram_tensor("woT", [G, E], f32, kind="ExternalInput").ap()
    bqd = nc.dram_tensor("bqd", [128, DC], f32, kind="ExternalInput").ap()
    bkd = nc.dram_tensor("bkd", [128, DC], f32, kind="ExternalInput").ap()
    maskb = nc.dram_tensor("maskb", [128, KC], f32, kind="ExternalInput").ap()
    out = nc.dram_tensor("out", [E, S], f32, kind="ExternalOutput").ap()  # transposed
    # reciprocal bounce scratch: one row per (head, q-tile)
    rscr = nc.dram_tensor("rscr", [HL * QT, 512], f32, kind="ExternalOutput").ap()

    def xstream(pool, src, lo, ncols, dt=bf16, tag="xstream"):
        t = pool.tile([128, EC, 512], dt, tag=tag)
        ap = src[:, lo : lo + ncols].rearrange("(ec p) s -> p ec s", p=128)
        if dt == f32r:
            ap = ap.bitcast(f32r)
        nc.sync.dma_start(t[:, :, :ncols], ap)
        return t

    with tile.TileContext(nc) as tc:
        with (
            tc.tile_pool(name="weights", bufs=1) as wpool,
            tc.tile_pool(name="persist", bufs=1) as ppool,
            tc.tile_pool(name="stream", bufs=3) as stream,
            tc.tile_pool(name="qtile", bufs=2) as qpool,
        ):
            kT_sb = ppool.tile([128, DC, S], bf16)
            v_sb = ppool.tile([128, KC, HL, DH + 1], f32r)
            wq_sb = wpool.tile([128, EC, G], bf16)
            wo_sb = wpool.tile([128, GC, E], f32r)
            bq_sb = wpool.tile([128, DC], f32)
            bk_sb = wpool.tile([128, DC], f32)
            mb_sb = wpool.tile([128, KC], f32)
            # ones column for the denominator fusion: preset whole tile, the
            # projection copies later overwrite cols 0..DH-1 of each head block
            nc.gpsimd.memset(v_sb[:].bitcast(f32), 1.0)

            # ---------------- phase 1: kT and v projections ----------------
            with (
                tc.tile_pool(name="wtmp", bufs=1) as wtmp,
                tc.tile_pool(name="vstream", bufs=2) as vstream,
                tc.tile_pool(name="pj_psum", bufs=4, space="PSUM") as pj_psum,
            ):
                wk_sb = wtmp.tile([128, EC, G], bf16)
                wv_sb = wtmp.tile([128, EC, G], f32r)
                nc.sync.dma_start(
                    wk_sb[:], wkT.rearrange("(ec p) g -> p ec g", p=128)
                )
                nc.sync.dma_start(bk_sb[:], bkd)
                nc.sync.dma_start(mb_sb[:], maskb)

                xk_ts = [xstream(stream, xkT, st * 512, 512) for st in range(1)]
                nc.sync.dma_start(
                    wv_sb[:], wvT.rearrange("(ec p) g -> p ec g", p=128).bitcast(f32r)
                )
                for st in range(QT):
                    xk_t = xk_ts[0] if st == 0 else xstream(stream, xkT, st * 512, 512)
                    for dc in range(DC):
                        ps = pj_psum.tile([128, 512], f32, tag="pj")
                        for ec in range(EC):
                            nc.tensor.matmul(
                                ps[:],
                                lhsT=wk_sb[:, ec, dc * 128 : (dc + 1) * 128],
                                rhs=xk_t[:, ec, :],
                                start=(ec == 0),
                                stop=(ec == EC - 1),
                            )
                        nc.vector.tensor_add(
                            out=kT_sb[:, dc, st * 512 : (st + 1) * 512],
                            in0=ps[:],
                            in1=bk_sb[:, dc : dc + 1].to_broadcast((128, 512)),
                        )

                for sg in range(S // 512):
                    xv_t = xstream(vstream, xvT, sg * 512, 512, dt=f32r, tag="xvstream")
                    for sci in range(4):
                        sc = sg * 4 + sci
                        ps = pj_psum.tile([128, 512], f32, tag="pj")
                        for ec in range(EC):
                            nc.tensor.matmul(
                                ps[:, :G],
                                lhsT=xv_t[:, ec, sci * 128 : (sci + 1) * 128],
                                rhs=wv_sb[:, ec, :],
                                start=(ec == 0),
                                stop=(ec == EC - 1),
                            )
                        nc.vector.tensor_copy(
                            out=v_sb[:, sc, :, 0:DH],
                            in_=ps[:, :G].rearrange("p (h d) -> p h d", h=HL),
                        )
                nc.sync.dma_start(bq_sb[:], bqd)
                nc.sync.dma_start(
                    wq_sb[:], wqT.rearrange("(ec p) g -> p ec g", p=128)
                )
                nc.sync.dma_start(
                    wo_sb[:], woT.rearrange("(gc p) e -> p gc e", p=128).bitcast(f32r)
                )

            # ---------------- phase 2: attention with interleaved proj/outproj ----------------
            ctxp_cm = tc.tile_pool(name="ctxp", bufs=1)
            ctxp = ctxp_cm.__enter__()
            ctxT_sb = ctxp.tile([128, DC, S], f32r)

            with (
                tc.tile_pool(name="exp", bufs=6) as epool,
                tc.tile_pool(name="norm", bufs=3) as npool,
                tc.tile_pool(name="outsb", bufs=4) as opool,
                tc.tile_pool(name="s_psum", bufs=2, space="PSUM") as s_psum,
                tc.tile_pool(name="c_psum", bufs=4, space="PSUM") as c_psum,
            ):
                qT_ts = {}
                xq_ts = {}

                def qproj_steps(qt, dc):
                    """one dc-chunk of the qT projection; yields every 2 matmuls"""
                    if dc == 0:
                        qT_ts[qt] = qpool.tile(
                            [128, DC, 512], bf16, tag="qT", name=f"qT{qt}"
                        )
                        xq_ts[qt] = xstream(stream, xqT, qt * 512, 512)
                    qT_t = qT_ts[qt]
                    ps = c_psum.tile([128, 512], f32, tag="ctx", name=f"qp{qt}_{dc}")
                    for ec in range(EC):
                        nc.tensor.matmul(
                            ps[:],
                            lhsT=wq_sb[:, ec, dc * 128 : (dc + 1) * 128],
                            rhs=xq_ts[qt][:, ec, :],
                            start=(ec == 0),
                            stop=(ec == EC - 1),
                        )
                        if ec % 2 == 1:
                            yield
                    nc.vector.tensor_add(
                        out=qT_t[:, dc, :],
                        in0=ps[:],
                        in1=bq_sb[:, dc : dc + 1].to_broadcast((128, 512)),
                    )

                def outproj_steps(st, ec):
                    """one ec-chunk of the transposed output projection; yields every 2 matmuls"""
                    ps = c_psum.tile([128, 512], f32, tag="ctx", name=f"op{st}_{ec}")
                    for gc in range(GC):
                        nc.tensor.matmul(
                            ps[:],
                            lhsT=wo_sb[:, gc, ec * 128 : (ec + 1) * 128],
                            rhs=ctxT_sb[:, gc, st * 512 : (st + 1) * 512],
                            start=(gc == 0),
                            stop=(gc == GC - 1),
                        )
                        if gc % 2 == 1:
                            yield
                    o_sb = opool.tile([128, 512], f32, tag="osb")
                    nc.vector.tensor_copy(out=o_sb[:], in_=ps[:])
                    nc.sync.dma_start(
                        out[ec * 128 : (ec + 1) * 128, st * 512 : (st + 1) * 512],
                        o_sb[:],
                    )

                def drive(bg, n=1):
                    """advance the background work queue by n yield-steps"""
                    while n > 0 and bg:
                        try:
                            next(bg[0])
                            n -= 1
                        except StopIteration:
                            bg.pop(0)

                for dc in range(DC):
                    for _ in qproj_steps(0, dc):
                        pass

                for qt in range(QT):
                    q0 = qt * 512
                    qT_t = qT_ts[qt]
                    for hp in range(DC):
                        bg = []
                        if qt < QT - 1:
                            bg.append(qproj_steps(qt + 1, hp))
                        if qt > 0:
                            bg.append(outproj_steps(qt - 1, 2 * hp))
                            bg.append(outproj_steps(qt - 1, 2 * hp + 1))
                        ctx0 = c_psum.tile([128, 512], f32, tag="ctx", name=f"c0_{qt}_{hp}")
                        ctx1 = c_psum.tile([128, 512], f32, tag="ctx", name=f"c1_{qt}_{hp}")
                        # software-pipelined: ctx(kc-1) and background work are
                        # emitted BEFORE the scores pair of kc so the scheduler
                        # keeps the two row-packed scores matmuls adjacent
                        pend = [None]

                        def ctx_pair(kc):
                            e = pend[0]
                            nc.tensor.matmul(
                                ctx0[0 : DH + 1, :],
                                lhsT=v_sb[:, kc, 2 * hp, :],
                                rhs=e[:, 0:512],
                                start=(kc == 0),
                                stop=(kc == KC - 1),
                            )
                            nc.tensor.matmul(
                                ctx1[0 : DH + 1, :],
                                lhsT=v_sb[:, kc, 2 * hp + 1, :],
                                rhs=e[:, 512:1024],
                                start=(kc == 0),
                                stop=(kc == KC - 1),
                            )

                        for kc in range(KC):
                            k0 = kc * 128
                            if kc > 0:
                                ctx_pair(kc - 1)
                            if kc % 2 == 1:
                                drive(bg, 1)
                            sp = s_psum.tile([128, 1024], f32, tag="sp")
                            nc.tensor.matmul(
                                sp[:, 0:512],
                                lhsT=kT_sb[0:64, hp, k0 : k0 + 128],
                                rhs=qT_t[0:64, hp, :],
                                start=True,
                                stop=True,
                            )
                            nc.tensor.matmul(
                                sp[:, 512:1024],
                                lhsT=kT_sb[64:128, hp, k0 : k0 + 128],
                                rhs=qT_t[64:128, hp, :],
                                start=True,
                                stop=True,
                            )
                            e = epool.tile([128, 1024], f32r, tag="exp")
                            nc.scalar.activation(
                                e[:], sp[:], AF.Exp,
                                bias=mb_sb[:, kc : kc + 1], scale=float(SCALE),
                            )
                            pend[0] = e
                        ctx_pair(KC - 1)
                        while bg:
                            drive(bg, 1)
                        # evacuate psum fast, then normalize in SBUF
                        for hq, cpsum in ((0, ctx0), (1, ctx1)):
                            pb = 64 * hq
                            qs = slice(q0, q0 + 512)
                            nc.vector.tensor_copy(
                                out=ctxT_sb[pb : pb + 64, hp, qs], in_=cpsum[0:DH, :]
                            )
                            den = npool.tile([1, 512], f32, tag="den")
                            nc.vector.tensor_copy(out=den[:], in_=cpsum[DH : DH + 1, :])
                            rec = npool.tile([1, 512], f32, tag="rec")
                            nc.vector.reciprocal_approx_fast(rec[:], den[:])
                            r = (2 * hp + hq) * QT + qt
                            nc.sync.dma_start(rscr[r : r + 1, :], rec[:])
                            rb = npool.tile([128, 512], f32, tag="rb")
                            nc.sync.dma_start(
                                rb[pb : pb + 64, :],
                                rscr[r : r + 1, :].to_broadcast((64, 512)),
                            )
                            nc.vector.tensor_mul(
                                out=ctxT_sb[pb : pb + 64, hp, qs],
                                in0=ctxT_sb[pb : pb + 64, hp, qs],
                                in1=rb[pb : pb + 64, :],
                            )

                # tail: output projection for the last q-tile
                for ec in range(EC):
                    for _ in outproj_steps(QT - 1, ec):
                        pass
            ctxp_cm.__exit__(None, None, None)

    nc.compile()
    return nc


def _prep_core_inputs(query, key, value, mask, Wq, bq, Wk, bk, Wv, Wo):
    """Per-core input maps: core c -> batch c//2, head-group c%2."""
    import ml_dtypes

    f = ml_dtypes.bfloat16
    maps = []
    for c in range(8):
        b, g = c // 2, c % 2
        lo = g * G
        mrow = mask[b, 0].astype(np.float64)
        maskb = np.where(mrow == 0, MASK_NEG, 0.0).reshape(KC, 128).T
        maps.append(
            {
                "xqT": np.ascontiguousarray(query[b].T).astype(f, copy=False),
                "xkT": np.ascontiguousarray(key[b].T).astype(f, copy=False),
                "xvT": np.ascontiguousarray(value[b].T).astype(np.float32, copy=False),
                "wqT": np.ascontiguousarray(Wq[lo : lo + G].T).astype(f, copy=False),
                "wkT": np.ascontiguousarray(Wk[lo : lo + G].T).astype(f, copy=False),
                "wvT": np.ascontiguousarray(Wv[lo : lo + G].T).astype(np.float32, copy=False),
                "woT": np.ascontiguousarray(Wo[:, lo : lo + G].T).astype(np.float32, copy=False),
                "bqd": np.ascontiguousarray(bq[lo : lo + G].reshape(DC, 128).T).astype(np.float32),
                "bkd": np.ascontiguousarray(bk[lo : lo + G].reshape(DC, 128).T).astype(np.float32),
                "maskb": np.ascontiguousarray(maskb).astype(np.float32),
            }
        )
    return maps


def kernel(query, key, value, mask, Wq, bq, Wk, bk, Wv, bv, Wo, bo, _results=None):
    global _NC
    query = np.asarray(query, dtype=np.float32)
    key = np.asarray(key, dtype=np.float32)
    value = np.asarray(value, dtype=np.float32)
    mask = np.asarray(mask)
    Wq, bq = np.asarray(Wq, np.float32), np.asarray(bq, np.float32)
    Wk, bk = np.asarray(Wk, np.float32), np.asarray(bk, np.float32)
    Wv, bv = np.asarray(Wv, np.float32), np.asarray(bv, np.float32)
    Wo, bo = np.asarray(Wo, np.float32), np.asarray(bo, np.float32)

    if _NC is None:
        _NC = _build_program()
    in_maps = _prep_core_inputs(query, key, value, mask, Wq, bq, Wk, bk, Wv, Wo)
    res = run_bass_kernel_spmd(_NC, in_maps, core_ids=list(range(8)))
    if _results is not None:
        _results.append(res)

    # host epilogue: sum the two head-group partials; bv commutes with softmax
    # (rows sum to 1) so its contribution is Wo @ bv, plus the output bias bo.
    extra = (Wo.astype(np.float64) @ bv.astype(np.float64) + bo.astype(np.float64)).astype(
        np.float32
    )
    out = np.empty((B, S, E), dtype=np.float32)
    for b in range(B):
        out[b] = (
            res.results[2 * b]["out"] + res.results[2 * b + 1]["out"]
        ).T + extra
    return out



# revision 11
# speedup vs baseline: 1.2313x; 1.2313x over previous
"""Multi-head attention (B=4, S=2048, D=1024, H=16) on 8 Trainium2 NeuronCores.

Sharding: core c handles batch c//2 and head-group c%2 (8 heads = 512 dims of
the per-head concat). Each core computes its q/k/v projections (tensor
parallel over heads), attention for its 8 heads, and a partial output
projection over its 512 concat dims; the host sums the two partials per batch.

Device dataflow (per core, all matmuls fp32r = full-rate PE):
  - qT/kT [d, s] layouts from projection (contraction over embedding on
    partitions), v in [s, d] layout with a fused ones-column per head.
  - scores computed transposed S^T[k, q] so the softmax mask/bias is a
    per-partition ACT bias and exp(scale*s + bias) is a single ACT op
    (no max subtraction needed: |scores| <= ~4 by construction).
  - ctx^T = [V | 1]^T @ P^T accumulated over k-chunks; row 64 of the psum is
    the softmax denominator (flash-style deferred normalization).
  - normalization: exact reciprocal of the denominator row, partition-broadcast
    via a DRAM bounce, one DVE multiply per head into ctx^T.
  - out^T partial = ctx_cat^T chunks @ Wo^T slices, streamed to DRAM.

Host epilogue: out[b] = partial[2b] + partial[2b+1] + (Wo @ bv + bo); the
value bias commutes with softmax (rows sum to 1) so it is exact. The key bias
is softmax-invariant (constant per query) and is still applied on-device for
exactness; so is the query bias.
"""

import sys

sys.path.insert(0, "/opt/trn_rl_repo")

import numpy as np

import concourse.bacc as bacc
import concourse.mybir as mybir
import concourse.tile as tile
from concourse.bass_utils import run_bass_kernel_spmd
import concourse.bass_utils as _bu

# Re-enable walrus LDWEIGHTS dedupe: fp32r matmuls are self-loading (no FWL,
# no background-buffer preload), so a repeated stationary operand otherwise
# pays a full serial weight load per matmul.
if False:
    _orig_run_command = _bu.run_command

    def _run_command_ldwopt(cmd, **kw):
        cmd = [
            "--enable-ldw-opt=true" if c == "--enable-ldw-opt=false" else c
            for c in cmd
        ]
        return _orig_run_command(cmd, **kw)

    _bu.run_command = _run_command_ldwopt
    _bu._ldwopt_patched = True

f32 = mybir.dt.float32
f32r = mybir.dt.float32r
bf16 = mybir.dt.bfloat16
AF = mybir.ActivationFunctionType

B, S, E, H = 4, 2048, 1024, 16
DH = E // H  # 64
G = E // 2  # 512 dims per core (8 heads)
HL = H // 2  # heads per core
EC = E // 128  # 8 e-chunks (projection contraction)
DC = G // 128  # 4 head-pairs per core
QT = S // 512  # 4 q-tiles
KC = S // 128  # 16 k-chunks
SC = S // 128  # 16 s-chunks (output rows)
GC = G // 128  # 4 chunks of the local concat dim (out-proj contraction)
SCALE = 1.0 / np.sqrt(np.float64(E))
MASK_NEG = -88.0  # exp(-88 + |s|max) == 0 in fp32 for masked keys

_NC = None


def _build_program():
    nc = bacc.Bacc("TRN2", target_bir_lowering=False, debug=False, num_devices=8)

    xqT = nc.dram_tensor("xqT", [E, S], bf16, kind="ExternalInput").ap()
    xkT = nc.dram_tensor("xkT", [E, S], bf16, kind="ExternalInput").ap()
    xvT = nc.dram_tensor("xvT", [E, S], bf16, kind="ExternalInput").ap()
    wqT = nc.dram_tensor("wqT", [E, G], bf16, kind="ExternalInput").ap()
    wkT = nc.dram_tensor("wkT", [E, G], bf16, kind="ExternalInput").ap()
    wvT = nc.dram_tensor("wvT", [E, G], bf16, kind="ExternalInput").ap()
    woT = nc.dram_tensor("woT", [G, E], bf16, kind="ExternalInput").ap()
    bqd = nc.dram_tensor("bqd", [128, DC], f32, kind="ExternalInput").ap()
    bkd = nc.dram_tensor("bkd", [128, DC], f32, kind="ExternalInput").ap()
    maskb = nc.dram_tensor("maskb", [128, KC], f32, kind="ExternalInput").ap()
    out = nc.dram_tensor("out", [E, S], f32, kind="ExternalOutput").ap()  # transposed
    # reciprocal bounce scratch: one row per (head, q-tile)
    rscr = nc.dram_tensor("rscr", [HL * QT, 512], f32, kind="ExternalOutput").ap()

    def xstream(pool, src, lo, ncols, dt=bf16, tag="xstream"):
        t = pool.tile([128, EC, 512], dt, tag=tag)
        ap = src[:, lo : lo + ncols].rearrange("(ec p) s -> p ec s", p=128)
        if dt == f32r:
            ap = ap.bitcast(f32r)
        nc.sync.dma_start(t[:, :, :ncols], ap)
        return t

    with tile.TileContext(nc) as tc:
        with (
            tc.tile_pool(name="weights", bufs=1) as wpool,
            tc.tile_pool(name="persist", bufs=1) as ppool,
            tc.tile_pool(name="stream", bufs=3) as stream,
            tc.tile_pool(name="qtile", bufs=2) as qpool,
        ):
            kT_sb = ppool.tile([128, DC, S], bf16)
            v_sb = ppool.tile([128, KC, HL, DH + 1], bf16)
            wq_sb = wpool.tile([128, EC, G], bf16)
            wo_sb = wpool.tile([128, GC, E], bf16)
            bq_sb = wpool.tile([128, DC], f32)
            bk_sb = wpool.tile([128, DC], f32)
            mb_sb = wpool.tile([128, KC], f32)
            # ones column for the denominator fusion: preset whole tile, the
            # projection copies later overwrite cols 0..DH-1 of each head block
            nc.gpsimd.memset(v_sb[:], 1.0)

            # ---------------- phase 1: kT and v projections ----------------
            with (
                tc.tile_pool(name="wtmp", bufs=1) as wtmp,
                tc.tile_pool(name="vstream", bufs=2) as vstream,
                tc.tile_pool(name="pj_psum", bufs=4, space="PSUM") as pj_psum,
            ):
                wk_sb = wtmp.tile([128, EC, G], bf16)
                wv_sb = wtmp.tile([128, EC, G], bf16)
                nc.sync.dma_start(
                    wk_sb[:], wkT.rearrange("(ec p) g -> p ec g", p=128)
                )
                nc.sync.dma_start(bk_sb[:], bkd)
                nc.sync.dma_start(mb_sb[:], maskb)

                xk_ts = [xstream(stream, xkT, st * 512, 512) for st in range(1)]
                nc.sync.dma_start(
                    wv_sb[:], wvT.rearrange("(ec p) g -> p ec g", p=128)
                )
                for st in range(QT):
                    xk_t = xk_ts[0] if st == 0 else xstream(stream, xkT, st * 512, 512)
                    for dc in range(DC):
                        ps = pj_psum.tile([128, 512], f32, tag="pj")
                        for ec in range(EC):
                            nc.tensor.matmul(
                                ps[:],
                                lhsT=wk_sb[:, ec, dc * 128 : (dc + 1) * 128],
                                rhs=xk_t[:, ec, :],
                                start=(ec == 0),
                                stop=(ec == EC - 1),
                            )
                        nc.vector.tensor_add(
                            out=kT_sb[:, dc, st * 512 : (st + 1) * 512],
                            in0=ps[:],
                            in1=bk_sb[:, dc : dc + 1].to_broadcast((128, 512)),
                        )

                for sg in range(S // 512):
                    xv_t = xstream(vstream, xvT, sg * 512, 512, dt=bf16, tag="xvstream")
                    for sci in range(4):
                        sc = sg * 4 + sci
                        ps = pj_psum.tile([128, 512], f32, tag="pj")
                        for ec in range(EC):
                            nc.tensor.matmul(
                                ps[:, :G],
                                lhsT=xv_t[:, ec, sci * 128 : (sci + 1) * 128],
                                rhs=wv_sb[:, ec, :],
                                start=(ec == 0),
                                stop=(ec == EC - 1),
                            )
                        nc.vector.tensor_copy(
                            out=v_sb[:, sc, :, 0:DH],
                            in_=ps[:, :G].rearrange("p (h d) -> p h d", h=HL),
                        )
                nc.sync.dma_start(bq_sb[:], bqd)
                nc.sync.dma_start(
                    wq_sb[:], wqT.rearrange("(ec p) g -> p ec g", p=128)
                )
                nc.sync.dma_start(
                    wo_sb[:], woT.rearrange("(gc p) e -> p gc e", p=128)
                )

            # ---------------- phase 2: attention with interleaved proj/outproj ----------------
            ctxp_cm = tc.tile_pool(name="ctxp", bufs=1)
            ctxp = ctxp_cm.__enter__()
            ctxT_sb = ctxp.tile([128, DC, S], bf16)

            with (
                tc.tile_pool(name="exp", bufs=6) as epool,
                tc.tile_pool(name="norm", bufs=3) as npool,
                tc.tile_pool(name="outsb", bufs=4) as opool,
                tc.tile_pool(name="s_psum", bufs=2, space="PSUM") as s_psum,
                tc.tile_pool(name="c_psum", bufs=4, space="PSUM") as c_psum,
            ):
                qT_ts = {}
                xq_ts = {}

                def qproj_steps(qt, dc):
                    """one dc-chunk of the qT projection; yields every 2 matmuls"""
                    if dc == 0:
                        qT_ts[qt] = qpool.tile(
                            [128, DC, 512], bf16, tag="qT", name=f"qT{qt}"
                        )
                        xq_ts[qt] = xstream(stream, xqT, qt * 512, 512)
                    qT_t = qT_ts[qt]
                    ps = c_psum.tile([128, 512], f32, tag="ctx", name=f"qp{qt}_{dc}")
                    for ec in range(EC):
                        nc.tensor.matmul(
                            ps[:],
                            lhsT=wq_sb[:, ec, dc * 128 : (dc + 1) * 128],
                            rhs=xq_ts[qt][:, ec, :],
                            start=(ec == 0),
                            stop=(ec == EC - 1),
                        )
                        if ec % 2 == 1:
                            yield
                    nc.vector.tensor_add(
                        out=qT_t[:, dc, :],
                        in0=ps[:],
                        in1=bq_sb[:, dc : dc + 1].to_broadcast((128, 512)),
                    )

                def outproj_steps(st, ec):
                    """one ec-chunk of the transposed output projection; yields every 2 matmuls"""
                    ps = c_psum.tile([128, 512], f32, tag="ctx", name=f"op{st}_{ec}")
                    for gc in range(GC):
                        nc.tensor.matmul(
                            ps[:],
                            lhsT=wo_sb[:, gc, ec * 128 : (ec + 1) * 128],
                            rhs=ctxT_sb[:, gc, st * 512 : (st + 1) * 512],
                            start=(gc == 0),
                            stop=(gc == GC - 1),
                        )
                        if gc % 2 == 1:
                            yield
                    o_sb = opool.tile([128, 512], f32, tag="osb")
                    nc.vector.tensor_copy(out=o_sb[:], in_=ps[:])
                    nc.sync.dma_start(
                        out[ec * 128 : (ec + 1) * 128, st * 512 : (st + 1) * 512],
                        o_sb[:],
                    )

                def drive(bg, n=1):
                    """advance the background work queue by n yield-steps"""
                    while n > 0 and bg:
                        try:
                            next(bg[0])
                            n -= 1
                        except StopIteration:
                            bg.pop(0)

                for dc in range(DC):
                    for _ in qproj_steps(0, dc):
                        pass

                for qt in range(QT):
                    q0 = qt * 512
                    qT_t = qT_ts[qt]
                    for hp in range(DC):
                        bg = []
                        if qt < QT - 1:
                            bg.append(qproj_steps(qt + 1, hp))
                        if qt > 0:
                            bg.append(outproj_steps(qt - 1, 2 * hp))
                            bg.append(outproj_steps(qt - 1, 2 * hp + 1))
                        ctx0 = c_psum.tile([128, 512], f32, tag="ctx", name=f"c0_{qt}_{hp}")
                        ctx1 = c_psum.tile([128, 512], f32, tag="ctx", name=f"c1_{qt}_{hp}")
                        # software-pipelined: ctx(kc-1) and background work are
                        # emitted BEFORE the scores pair of kc so the scheduler
                        # keeps the two row-packed scores matmuls adjacent
                        pend = [None]

                        def ctx_pair(kc):
                            e = pend[0]
                            nc.tensor.matmul(
                                ctx0[0 : DH + 1, :],
                                lhsT=v_sb[:, kc, 2 * hp, :],
                                rhs=e[:, 0:512],
                                start=(kc == 0),
                                stop=(kc == KC - 1),
                            )
                            nc.tensor.matmul(
                                ctx1[0 : DH + 1, :],
                                lhsT=v_sb[:, kc, 2 * hp + 1, :],
                                rhs=e[:, 512:1024],
                                start=(kc == 0),
                                stop=(kc == KC - 1),
                            )

                        for kc in range(KC):
                            k0 = kc * 128
                            if kc > 0:
                                ctx_pair(kc - 1)
                            if kc % 2 == 1:
                                drive(bg, 1)
                            sp = s_psum.tile([128, 1024], f32, tag="sp")
                            nc.tensor.matmul(
                                sp[:, 0:512],
                                lhsT=kT_sb[0:64, hp, k0 : k0 + 128],
                                rhs=qT_t[0:64, hp, :],
                                start=True,
                                stop=True,
                            )
                            nc.tensor.matmul(
                                sp[:, 512:1024],
                                lhsT=kT_sb[64:128, hp, k0 : k0 + 128],
                                rhs=qT_t[64:128, hp, :],
                                start=True,
                                stop=True,
                            )
                            e = epool.tile([128, 1024], bf16, tag="exp")
                            nc.scalar.activation(
                                e[:], sp[:], AF.Exp,
                                bias=mb_sb[:, kc : kc + 1], scale=float(SCALE),
                            )
                            pend[0] = e
                        ctx_pair(KC - 1)
                        while bg:
                            drive(bg, 1)
                        # evacuate psum fast, then normalize in SBUF
                        for hq, cpsum in ((0, ctx0), (1, ctx1)):
                            pb = 64 * hq
                            qs = slice(q0, q0 + 512)
                            nc.vector.tensor_copy(
                                out=ctxT_sb[pb : pb + 64, hp, qs], in_=cpsum[0:DH, :]
                            )
                            den = npool.tile([1, 512], f32, tag="den")
                            nc.vector.tensor_copy(out=den[:], in_=cpsum[DH : DH + 1, :])
                            rec = npool.tile([1, 512], f32, tag="rec")
                            nc.vector.reciprocal_approx_fast(rec[:], den[:])
                            r = (2 * hp + hq) * QT + qt
                            nc.sync.dma_start(rscr[r : r + 1, :], rec[:])
                            rb = npool.tile([128, 512], f32, tag="rb")
                            nc.sync.dma_start(
                                rb[pb : pb + 64, :],
                                rscr[r : r + 1, :].to_broadcast((64, 512)),
                            )
                            nc.vector.tensor_mul(
                                out=ctxT_sb[pb : pb + 64, hp, qs],
                                in0=ctxT_sb[pb : pb + 64, hp, qs],
                                in1=rb[pb : pb + 64, :],
                            )

                # tail: output projection for the last q-tile
                for ec in range(EC):
                    for _ in outproj_steps(QT - 1, ec):
                        pass
            ctxp_cm.__exit__(None, None, None)

    nc.compile()
    return nc


def _prep_core_inputs(query, key, value, mask, Wq, bq, Wk, bk, Wv, Wo):
    """Per-core input maps: core c -> batch c//2, head-group c%2."""
    import ml_dtypes

    f = ml_dtypes.bfloat16
    maps = []
    for c in range(8):
        b, g = c // 2, c % 2
        lo = g * G
        mrow = mask[b, 0].astype(np.float64)
        maskb = np.where(mrow == 0, MASK_NEG, 0.0).reshape(KC, 128).T
        maps.append(
            {
                "xqT": np.ascontiguousarray(query[b].T).astype(f, copy=False),
                "xkT": np.ascontiguousarray(key[b].T).astype(f, copy=False),
                "xvT": np.ascontiguousarray(value[b].T).astype(f, copy=False),
                "wqT": np.ascontiguousarray(Wq[lo : lo + G].T).astype(f, copy=False),
                "wkT": np.ascontiguousarray(Wk[lo : lo + G].T).astype(f, copy=False),
                "wvT": np.ascontiguousarray(Wv[lo : lo + G].T).astype(f, copy=False),
                "woT": np.ascontiguousarray(Wo[:, lo : lo + G].T).astype(f, copy=False),
                "bqd": np.ascontiguousarray(bq[lo : lo + G].reshape(DC, 128).T).astype(np.float32),
                "bkd": np.ascontiguousarray(bk[lo : lo + G].reshape(DC, 128).T).astype(np.float32),
                "maskb": np.ascontiguousarray(maskb).astype(np.float32),
            }
        )
    return maps


def kernel(query, key, value, mask, Wq, bq, Wk, bk, Wv, bv, Wo, bo, _results=None):
    global _NC
    query = np.asarray(query, dtype=np.float32)
    key = np.asarray(key, dtype=np.float32)
    value = np.asarray(value, dtype=np.float32)
    mask = np.asarray(mask)
    Wq, bq = np.asarray(Wq, np.float32), np.asarray(bq, np.float32)
    Wk, bk = np.asarray(Wk, np.float32), np.asarray(bk, np.float32)
    Wv, bv = np.asarray(Wv, np.float32), np.asarray(bv, np.float32)
    Wo, bo = np.asarray(Wo, np.float32), np.asarray(bo, np.float32)

    if _NC is None:
        _NC = _build_program()
    in_maps = _prep_core_inputs(query, key, value, mask, Wq, bq, Wk, bk, Wv, Wo)
    res = run_bass_kernel_spmd(_NC, in_maps, core_ids=list(range(8)))
    if _results is not None:
        _results.append(res)

    # host epilogue: sum the two head-group partials; bv commutes with softmax
    # (rows sum to 1) so its contribution is Wo @ bv, plus the output bias bo.
    extra = (Wo.astype(np.float64) @ bv.astype(np.float64) + bo.astype(np.float64)).astype(
        np.float32
    )
    out = np.empty((B, S, E), dtype=np.float32)
    for b in range(B):
        out[b] = (
            res.results[2 * b]["out"] + res.results[2 * b + 1]["out"]
        ).T + extra
    return out



# revision 17
# speedup vs baseline: 1.3268x; 1.0775x over previous
"""Multi-head attention (B=4, S=2048, D=1024, H=16) on 8 Trainium2 NeuronCores.

Sharding: core c handles batch c//2 and head-group c%2 (8 heads = 512 dims of
the per-head concat). Each core computes its q/k/v projections (tensor
parallel over heads), attention for its 8 heads, and a partial output
projection over its 512 concat dims; the host sums the two partials per batch.

v3 dataflow (all matmuls bf16, f32 psum accumulate):
  - scores computed transposed S^T[k, q] so the softmax mask/bias is a
    per-partition ACT bias and exp(scale*s + bias) is one ACT op; the two
    K=64 head-halves run row-packed (concurrent row groups).
  - ctx^T = [V | 1]^T @ e accumulated over k-chunks, M=65: psum row 64 is
    the softmax denominator (ones column rides the contraction for free).
  - ctx matmuls lag the exp by TWO k-chunks so the PE stream never waits
    on the ACT semaphore (scores for kc are emitted ahead of ctx(kc-2)).
  - k/v/q projections and the transposed output projection run as
    background generators inside the attention loop; unit closes
    (normalization) are deferred into the next unit's first k-chunks so
    the ACT queue never drains at unit boundaries.
  - exp instructions are the serial resource: 256 x [128,1024] on the ACT
    engine (~1.1us each) bound the kernel; everything else hides under it.

PSUM banks: scores 2x[128,1024]=4, ctx pair (M=65) 2, proj 2.

Host epilogue: out[b] = partial[2b] + partial[2b+1] + (Wo @ bv + bo); the
value bias commutes with softmax (rows sum to 1) so it is exact. Key/query
biases applied on-device.
"""

import sys

sys.path.insert(0, "/opt/trn_rl_repo")

import numpy as np

import concourse.bacc as bacc
import concourse.mybir as mybir
import concourse.tile as tile
from concourse.bass_utils import run_bass_kernel_spmd

f32 = mybir.dt.float32
bf16 = mybir.dt.bfloat16
AF = mybir.ActivationFunctionType

B, S, E, H = 4, 2048, 1024, 16
DH = E // H  # 64
G = E // 2  # 512 dims per core (8 heads)
HL = H // 2  # heads per core
EC = E // 128  # 8 e-chunks (projection contraction)
DC = G // 128  # 4 head-pairs per core
QT = S // 512  # 4 q-tiles
KC = S // 128  # 16 k-chunks
GC = G // 128  # 4 chunks of the local concat dim (out-proj contraction)
SCALE = 1.0 / np.sqrt(np.float64(E))
MASK_NEG = -88.0  # exp(-88 + |s|max) == 0 in fp32 for masked keys

_NC = None


def _build_program():
    nc = bacc.Bacc("TRN2", target_bir_lowering=False, debug=False, num_devices=8)

    xqT = nc.dram_tensor("xqT", [E, S], bf16, kind="ExternalInput").ap()
    xkT = nc.dram_tensor("xkT", [E, S], bf16, kind="ExternalInput").ap()
    xvT = nc.dram_tensor("xvT", [E, S], bf16, kind="ExternalInput").ap()
    wqT = nc.dram_tensor("wqT", [E, G], bf16, kind="ExternalInput").ap()
    wkT = nc.dram_tensor("wkT", [E, G], bf16, kind="ExternalInput").ap()
    wvT = nc.dram_tensor("wvT", [E, G], bf16, kind="ExternalInput").ap()
    woT = nc.dram_tensor("woT", [G, E], bf16, kind="ExternalInput").ap()
    bqd = nc.dram_tensor("bqd", [128, DC], f32, kind="ExternalInput").ap()
    bkd = nc.dram_tensor("bkd", [128, DC], f32, kind="ExternalInput").ap()
    maskb = nc.dram_tensor("maskb", [128, KC], f32, kind="ExternalInput").ap()
    out = nc.dram_tensor("out", [E, S], f32, kind="ExternalOutput").ap()  # transposed
    # reciprocal bounce scratch: one row per (head, q-tile)
    rscr = nc.dram_tensor("rscr", [HL * QT, 512], f32, kind="ExternalOutput").ap()

    with tile.TileContext(nc) as tc:
        with (
            tc.tile_pool(name="weights", bufs=1) as wpool,
            tc.tile_pool(name="persist", bufs=1) as ppool,
            tc.tile_pool(name="xkstream", bufs=4) as xkstream,
            tc.tile_pool(name="xvstream", bufs=2) as xvstream,
            tc.tile_pool(name="xqstream", bufs=2) as xqstream,
            tc.tile_pool(name="qtile", bufs=2) as qpool,
            tc.tile_pool(name="exp", bufs=6) as epool,
            tc.tile_pool(name="norm", bufs=4) as npool,
            tc.tile_pool(name="outsb", bufs=3) as opool,
            tc.tile_pool(name="s_psum", bufs=2, space="PSUM") as s_psum,
            tc.tile_pool(name="c_psum", bufs=2, space="PSUM") as c_psum,
            tc.tile_pool(name="p_psum", bufs=2, space="PSUM") as p_psum,
        ):
            kT_sb = ppool.tile([128, DC, S], bf16)
            v_sb = ppool.tile([128, KC, HL, DH + 1], bf16)
            ctxT_sb = ppool.tile([128, DC, S], bf16)
            wq_sb = wpool.tile([128, EC, G], bf16)
            wk_sb = wpool.tile([128, EC, G], bf16)
            wv_sb = wpool.tile([128, EC, G], bf16)
            wo_sb = wpool.tile([128, GC, E], bf16)
            bq_sb = wpool.tile([128, DC], f32)
            bk_sb = wpool.tile([128, DC], f32)
            mb_sb = wpool.tile([128, KC], f32)

            # ones column for the denominator fusion: preset the whole tile,
            # the projection copies later overwrite cols 0..DH-1 per head
            nc.gpsimd.memset(v_sb[:], 1.0)

            def xstream(pool, src, lo, tag):
                t = pool.tile([128, EC, 512], bf16, tag=tag)
                nc.sync.dma_start(
                    t[:], src[:, lo : lo + 512].rearrange("(ec p) s -> p ec s", p=128)
                )
                return t

            # DMAs in need-order: k-projection inputs first, wo last
            nc.sync.dma_start(bk_sb[:], bkd)
            nc.sync.dma_start(mb_sb[:], maskb)
            nc.sync.dma_start(bq_sb[:], bqd)
            nc.sync.dma_start(wk_sb[:], wkT.rearrange("(ec p) g -> p ec g", p=128))
            xk_ts = {0: xstream(xkstream, xkT, 0, "xk")}
            nc.sync.dma_start(wv_sb[:], wvT.rearrange("(ec p) g -> p ec g", p=128))
            xv_t0 = xstream(xvstream, xvT, 0, "xv")
            nc.sync.dma_start(wq_sb[:], wqT.rearrange("(ec p) g -> p ec g", p=128))
            nc.sync.dma_start(wo_sb[:], woT.rearrange("(gc p) e -> p gc e", p=128))

            # ---------- background work generators (yield ~per matmul) ----------
            def kproj_chunk(st, dc):
                ps = p_psum.tile([128, 512], f32, tag="proj", name=f"kp{st}_{dc}")
                for ec in range(EC):
                    nc.tensor.matmul(
                        ps[:],
                        lhsT=wk_sb[:, ec, dc * 128 : (dc + 1) * 128],
                        rhs=xk_ts[st][:, ec, :],
                        start=(ec == 0),
                        stop=(ec == EC - 1),
                    )
                    yield
                nc.vector.tensor_add(
                    out=kT_sb[:, dc, st * 512 : (st + 1) * 512],
                    in0=ps[:],
                    in1=bk_sb[:, dc : dc + 1].to_broadcast((128, 512)),
                )

            def vproj_chunk(xv_t, st, sci):
                sc = st * 4 + sci
                ps = p_psum.tile([128, 512], f32, tag="proj", name=f"vp{sc}")
                for ec in range(EC):
                    nc.tensor.matmul(
                        ps[:, :G],
                        lhsT=xv_t[:, ec, sci * 128 : (sci + 1) * 128],
                        rhs=wv_sb[:, ec, :],
                        start=(ec == 0),
                        stop=(ec == EC - 1),
                    )
                    yield
                nc.vector.tensor_copy(
                    out=v_sb[:, sc, :, 0:DH],
                    in_=ps[:, :G].rearrange("p (h d) -> p h d", h=HL),
                )

            qT_ts = {}
            xq_ts = {}

            def qproj_chunk(qt, dc):
                if qt not in qT_ts:
                    qT_ts[qt] = qpool.tile([128, DC, 512], bf16, tag="qT", name=f"qT{qt}")
                    xq_ts[qt] = xstream(xqstream, xqT, qt * 512, "xq")
                ps = p_psum.tile([128, 512], f32, tag="proj", name=f"qp{qt}_{dc}")
                for ec in range(EC):
                    nc.tensor.matmul(
                        ps[:],
                        lhsT=wq_sb[:, ec, dc * 128 : (dc + 1) * 128],
                        rhs=xq_ts[qt][:, ec, :],
                        start=(ec == 0),
                        stop=(ec == EC - 1),
                    )
                    yield
                nc.vector.tensor_add(
                    out=qT_ts[qt][:, dc, :],
                    in0=ps[:],
                    in1=bq_sb[:, dc : dc + 1].to_broadcast((128, 512)),
                )

            def outproj_chunk(st, ec, ps=None):
                if ps is None:
                    ps = p_psum.tile([128, 512], f32, tag="proj", name=f"op{st}_{ec}")[:]
                for gc in range(GC):
                    nc.tensor.matmul(
                        ps,
                        lhsT=wo_sb[:, gc, ec * 128 : (ec + 1) * 128],
                        rhs=ctxT_sb[:, gc, st * 512 : (st + 1) * 512],
                        start=(gc == 0),
                        stop=(gc == GC - 1),
                    )
                    yield
                o_sb = opool.tile([128, 512], f32, tag="osb")
                nc.vector.tensor_copy(out=o_sb[:], in_=ps)
                nc.sync.dma_start(
                    out[ec * 128 : (ec + 1) * 128, st * 512 : (st + 1) * 512],
                    o_sb[:],
                )

            bg = []

            def drive(n=1):
                while n > 0 and bg:
                    try:
                        next(bg[0])
                        n -= 1
                    except StopIteration:
                        bg.pop(0)

            def drain_all():
                while bg:
                    drive(1)

            def force(g):
                while True:
                    try:
                        next(g)
                    except StopIteration:
                        break
                if g in bg:
                    bg.remove(g)

            # ---------- attention unit machinery (all state keyed per unit) ----
            ctx_ps = {}  # (qt, hp, hq) -> psum tile
            e_tiles = {}  # (qt, hp, kc) -> e tile
            pendq = {}  # (qt, hp) -> kcs whose ctx is not yet emitted (lag 2)

            def ctx_step(qt, hp, kc):
                """emit the M=65 ctx pair for kc (consumes its e tile)"""
                e = e_tiles.pop((qt, hp, kc))
                for hq in range(2):
                    if (qt, hp, hq) not in ctx_ps:
                        ctx_ps[(qt, hp, hq)] = c_psum.tile(
                            [128, 512], f32, tag="ctx", name=f"c{qt}_{hp}_{hq}"
                        )
                    nc.tensor.matmul(
                        ctx_ps[(qt, hp, hq)][0 : DH + 1, :],
                        lhsT=v_sb[:, kc, 2 * hp + hq, :],
                        rhs=e[:, 512 * hq : 512 * hq + 512],
                        start=(kc == 0),
                        stop=(kc == KC - 1),
                    )

            def att_kc(qt, hp, kc, bg_steps=2):
                """scores + exp for kc; ctx for kc-2."""
                qT_t = qT_ts[qt]
                k0 = kc * 128
                sp = s_psum.tile([128, 1024], f32, tag="sp")
                nc.tensor.matmul(
                    sp[:, 0:512],
                    lhsT=kT_sb[0:64, hp, k0 : k0 + 128],
                    rhs=qT_t[0:64, hp, :],
                    start=True,
                    stop=True,
                )
                nc.tensor.matmul(
                    sp[:, 512:1024],
                    lhsT=kT_sb[64:128, hp, k0 : k0 + 128],
                    rhs=qT_t[64:128, hp, :],
                    start=True,
                    stop=True,
                )
                e = epool.tile([128, 1024], bf16, tag="exp")
                nc.scalar.activation(
                    e[:], sp[:], AF.Exp,
                    bias=mb_sb[:, kc : kc + 1], scale=float(SCALE),
                )
                e_tiles[(qt, hp, kc)] = e
                q = pendq.setdefault((qt, hp), [])
                q.append(kc)
                if len(q) > 2:
                    ctx_step(qt, hp, q.pop(0))
                drive(bg_steps)

            def att_close(qt, hp):
                """final ctx steps, normalization, ctx evacuation."""
                q = pendq.pop((qt, hp))
                while q:
                    ctx_step(qt, hp, q.pop(0))
                qs = slice(qt * 512, qt * 512 + 512)
                rb = npool.tile([128, 512], f32, tag="rb")
                for hq in range(2):
                    cp = ctx_ps.pop((qt, hp, hq))
                    # evacuate ctx (frees the bank) then normalize in SBUF
                    nc.vector.tensor_copy(
                        out=ctxT_sb[64 * hq : 64 * hq + 64, hp, qs], in_=cp[0:DH, :]
                    )
                    den = npool.tile([1, 512], f32, tag="den")
                    nc.vector.tensor_copy(out=den[:], in_=cp[DH : DH + 1, :])
                    rec = npool.tile([1, 512], f32, tag="rec")
                    nc.vector.reciprocal_approx_fast(rec[:], den[:])
                    r = (2 * hp + hq) * QT + qt
                    nc.sync.dma_start(rscr[r : r + 1, :], rec[:])
                    nc.sync.dma_start(
                        rb[64 * hq : 64 * hq + 64, :],
                        rscr[r : r + 1, :].to_broadcast((64, 512)),
                    )
                nc.vector.tensor_mul(
                    out=ctxT_sb[:, hp, qs],
                    in0=ctxT_sb[:, hp, qs],
                    in1=rb[:],
                )

            # ---------- schedule ----------
            # prologue: wave 0 of k/v projections + q projection for hp 0
            for dc in range(DC):
                for _ in kproj_chunk(0, dc):
                    pass
            for sci in range(4):
                for _ in vproj_chunk(xv_t0, 0, sci):
                    pass
            qp_gens = {(0, dc): qproj_chunk(0, dc) for dc in range(DC)}
            force(qp_gens.pop((0, 0)))
            # waves 1..3: vproj + per-dc kproj, forced as late as possible
            wave_vp = {}
            wave_kp = {}
            for st in range(1, QT):
                xk_ts[st] = xstream(xkstream, xkT, st * 512, "xk")
                xv_s = xstream(xvstream, xvT, st * 512, "xv")
                wave_kp[st] = {dc: kproj_chunk(st, dc) for dc in range(DC)}
                wave_vp[st] = [vproj_chunk(xv_s, st, sci) for sci in range(4)]
                bg.append(wave_kp[st][0])
                bg.extend(wave_vp[st])
            for st in range(1, QT):
                for dc in range(1, DC):
                    bg.append(wave_kp[st][dc])
            bg.append(qp_gens[(0, 1)])
            bg.append(qp_gens[(0, 2)])
            bg.append(qp_gens[(0, 3)])

            def ensure_wave(st, hp):
                """emit everything attention (qt0, hp) needs for k-chunks of st"""
                if st == 0:
                    return
                for g in wave_vp[st]:
                    force(g)
                for dc in range(hp + 1):
                    force(wave_kp[st][dc])

            def unit_begin(qt, hp):
                g = qp_gens.pop((qt, hp), None)
                if g is not None:
                    force(g)

            # closes deferred into the next unit's first k-chunks
            closes = []

            def pop_close():
                if closes:
                    att_close(*closes.pop(0))

            for qt in range(QT):
                for hp in range(DC):
                    unit_begin(qt, hp)
                    if qt >= 1 and hp == 1:
                        # qt-1 rows of ctxT are final: output projection
                        for hh in range(DC):
                            bg.append(outproj_chunk(qt - 1, 2 * hh))
                            bg.append(outproj_chunk(qt - 1, 2 * hh + 1))
                    for kc in range(KC):
                        if qt == 0 and kc % 4 == 0:
                            ensure_wave(kc // 4, hp)
                        att_kc(qt, hp, kc, bg_steps=2)
                        if kc <= 1:
                            pop_close()
                    closes.append((qt, hp))
                    if qt < QT - 1:
                        qp_gens[(qt + 1, hp)] = qproj_chunk(qt + 1, hp)
                        bg.append(qp_gens[(qt + 1, hp)])
            while closes:
                pop_close()

            # tail: final output projection over rotating psum banks
            drain_all()
            s1 = s_psum.tile([128, 1024], f32, tag="sp", name="tail1")
            s2 = s_psum.tile([128, 1024], f32, tag="sp", name="tail2")
            tail_ps = [
                p_psum.tile([128, 512], f32, tag="proj", name="tp0")[:],
                p_psum.tile([128, 512], f32, tag="proj", name="tp1")[:],
                c_psum.tile([128, 512], f32, tag="ctx", name="tc0")[:],
                c_psum.tile([128, 512], f32, tag="ctx", name="tc1")[:],
                s1[:, 0:512],
                s1[:, 512:1024],
                s2[:, 0:512],
                s2[:, 512:1024],
            ]
            tail_gens = [
                outproj_chunk(QT - 1, ec, ps=tail_ps[ec]) for ec in range(EC)
            ]
            alive = list(tail_gens)
            while alive:
                for g in list(alive):
                    try:
                        next(g)
                    except StopIteration:
                        alive.remove(g)

    nc.compile()
    return nc


def _prep_core_inputs(query, key, value, mask, Wq, bq, Wk, bk, Wv, Wo):
    """Per-core input maps: core c -> batch c//2, head-group c%2."""
    import ml_dtypes

    f = ml_dtypes.bfloat16
    maps = []
    for c in range(8):
        b, g = c // 2, c % 2
        lo = g * G
        mrow = mask[b, 0].astype(np.float64)
        maskb = np.where(mrow == 0, MASK_NEG, 0.0).reshape(KC, 128).T
        maps.append(
            {
                "xqT": np.ascontiguousarray(query[b].T).astype(f, copy=False),
                "xkT": np.ascontiguousarray(key[b].T).astype(f, copy=False),
                "xvT": np.ascontiguousarray(value[b].T).astype(f, copy=False),
                "wqT": np.ascontiguousarray(Wq[lo : lo + G].T).astype(f, copy=False),
                "wkT": np.ascontiguousarray(Wk[lo : lo + G].T).astype(f, copy=False),
                "wvT": np.ascontiguousarray(Wv[lo : lo + G].T).astype(f, copy=False),
                "woT": np.ascontiguousarray(Wo[:, lo : lo + G].T).astype(f, copy=False),
                "bqd": np.ascontiguousarray(bq[lo : lo + G].reshape(DC, 128).T).astype(np.float32),
                "bkd": np.ascontiguousarray(bk[lo : lo + G].reshape(DC, 128).T).astype(np.float32),
                "maskb": np.ascontiguousarray(maskb).astype(np.float32),
            }
        )
    return maps


def kernel(query, key, value, mask, Wq, bq, Wk, bk, Wv, bv, Wo, bo, _results=None):
    global _NC
    query = np.asarray(query, dtype=np.float32)
    key = np.asarray(key, dtype=np.float32)
    value = np.asarray(value, dtype=np.float32)
    mask = np.asarray(mask)
    Wq, bq = np.asarray(Wq, np.float32), np.asarray(bq, np.float32)
    Wk, bk = np.asarray(Wk, np.float32), np.asarray(bk, np.float32)
    Wv, bv = np.asarray(Wv, np.float32), np.asarray(bv, np.float32)
    Wo, bo = np.asarray(Wo, np.float32), np.asarray(bo, np.float32)

    if _NC is None:
        _NC = _build_program()
    in_maps = _prep_core_inputs(query, key, value, mask, Wq, bq, Wk, bk, Wv, Wo)
    res = run_bass_kernel_spmd(_NC, in_maps, core_ids=list(range(8)))
    if _results is not None:
        _results.append(res)

    # host epilogue: sum the two head-group partials; bv commutes with softmax
    # (rows sum to 1) so its contribution is Wo @ bv, plus the output bias bo.
    extra = (Wo.astype(np.float64) @ bv.astype(np.float64) + bo.astype(np.float64)).astype(
        np.float32
    )
    out = np.empty((B, S, E), dtype=np.float32)
    for b in range(B):
        out[b] = (
            res.results[2 * b]["out"] + res.results[2 * b + 1]["out"]
        ).T + extra
    return out
